# revision 1
# baseline (speedup 1.0000x reference)
"""GPT forward pass on 8 Trainium2 NeuronCores.

Sharding: token-parallel trunk. Core c owns q-tile c (rows 128c..128c+127)
of each of the 4 sequences (512 tokens/core). Attention needs all K/V, which
is AllGathered (bf16) across the 8 cores once per layer. The lm_head is
vocab-sharded (4000 cols/core) over an AllGather of the final hidden states.
All matmuls run in bf16 with fp32 PSUM accumulation; layernorm/softmax
statistics and residual stream stay fp32.

Softmax is computed in transposed layout: S^T[k,q] = (K^T).T @ Q^T, exp on
ScalarE, multiplicative causal mask on VectorE, and the denominators come for
free from the P@V matmul by appending a ones-column to V.
"""

import os
import sys

for _p in ("/opt/trn_rl_repo",):
    if os.path.isdir(_p) and _p not in sys.path:
        sys.path.insert(0, _p)

import numpy as np
import ml_dtypes

BF16NP = ml_dtypes.bfloat16

import concourse.bass as bass
import concourse.mybir as mybir
import concourse.tile as tile
from concourse import bacc
from concourse.bass_utils import run_bass_kernel_spmd
from concourse.masks import make_identity

F32 = mybir.dt.float32
BF = mybir.dt.bfloat16
AF = mybir.ActivationFunctionType

V, C, T, H, L, B = 32000, 1024, 1024, 16, 4, 4
HD = C // H          # 64
FF = 4 * C           # 4096
NCORES = 8
TL = 512             # local tokens per core (4 seqs x 128)
SEQ = B              # 4
NT = TL // 128       # 4  local t-tiles; tile tt holds seq tt rows
NCT = C // 128       # 8  c-tiles
NFT = FF // 128      # 32 f-tiles
VSH = V // NCORES    # 4000 vocab shard
NVC = 8
VCW = VSH // NVC     # 500
LN_EPS = 1e-5

KV_K = C * TL        # elems in K^T region of kv_loc
KV_SZ = 2 * C * TL   # elems per-core kv payload (K^T + V)

_prog_cache = {}


def _ap(t, offset, pattern):
    return bass.AP(tensor=t.tensor if isinstance(t, bass.AP) else t, offset=offset, ap=pattern)


def _build(LL=L, debug=False, sim=False):
    key = (LL, debug, sim)
    if key in _prog_cache:
        return _prog_cache[key]

    nc = bacc.Bacc("TRN2", target_bir_lowering=False, debug=False, num_devices=NCORES)

    x0 = nc.dram_tensor("x0", [TL, C], F32, kind="ExternalInput")
    maskT_d = nc.dram_tensor("maskT", [128, NCT, 128], BF, kind="ExternalInput")
    wq_d = nc.dram_tensor("wq", [L, C, C], BF, kind="ExternalInput")
    wk_d = nc.dram_tensor("wk", [L, C, C], BF, kind="ExternalInput")
    wv_d = nc.dram_tensor("wv", [L, C, C], BF, kind="ExternalInput")
    wo_d = nc.dram_tensor("wo", [L, C, C], BF, kind="ExternalInput")
    w1_d = nc.dram_tensor("w1", [L, C, FF], BF, kind="ExternalInput")
    w2_d = nc.dram_tensor("w2", [L, FF, C], BF, kind="ExternalInput")
    b1_d = nc.dram_tensor("b1", [L, FF], F32, kind="ExternalInput")
    bo_d = nc.dram_tensor("bo", [L, C], F32, kind="ExternalInput")
    b2_d = nc.dram_tensor("b2", [L, C], F32, kind="ExternalInput")
    ln1g_d = nc.dram_tensor("ln1g", [L, C], F32, kind="ExternalInput")
    ln1b_d = nc.dram_tensor("ln1b", [L, C], F32, kind="ExternalInput")
    ln2g_d = nc.dram_tensor("ln2g", [L, C], F32, kind="ExternalInput")
    ln2b_d = nc.dram_tensor("ln2b", [L, C], F32, kind="ExternalInput")
    lnfg_d = nc.dram_tensor("lnfg", [C], F32, kind="ExternalInput")
    lnfb_d = nc.dram_tensor("lnfb", [C], F32, kind="ExternalInput")
    wlm_d = nc.dram_tensor("wlm", [C, VSH], BF, kind="ExternalInput")
    blm_d = nc.dram_tensor("blm", [VSH], BF, kind="ExternalInput")

    logits_d = nc.dram_tensor("logits", [NCORES * TL, VSH], F32, kind="ExternalOutput")
    dbg_d = None
    if debug:
        dbg_d = nc.dram_tensor("dbg", [LL, TL, C], F32, kind="ExternalOutput")

    with tile.TileContext(nc) as tc:
        import contextlib

        with contextlib.ExitStack() as ctx:
            # SBUF pools (per-partition KB in comments)
            const = ctx.enter_context(tc.tile_pool(name="const", bufs=1))      # ~2.5
            xpool = ctx.enter_context(tc.tile_pool(name="x", bufs=1))          # 16
            hpool = ctx.enter_context(tc.tile_pool(name="h", bufs=5))          # 10
            tpool = ctx.enter_context(tc.tile_pool(name="hT", bufs=1))         # 8
            qtpool = ctx.enter_context(tc.tile_pool(name="qt", bufs=1))        # 8
            kvpool = ctx.enter_context(tc.tile_pool(name="kv", bufs=4))        # 4
            otpool = ctx.enter_context(tc.tile_pool(name="oT", bufs=1))        # 8
            big = ctx.enter_context(tc.tile_pool(name="big", bufs=2))          # 64
            wpool = ctx.enter_context(tc.tile_pool(name="w", bufs=6))          # 6
            gbpool = ctx.enter_context(tc.tile_pool(name="gb", bufs=1))        # 16
            misc = ctx.enter_context(tc.tile_pool(name="misc", bufs=2))        # ~1
            lntmp = ctx.enter_context(tc.tile_pool(name="lntmp", bufs=2))      # 8
            kts_pool = ctx.enter_context(tc.tile_pool(name="kts", bufs=3))     # 4
            pt_pool = ctx.enter_context(tc.tile_pool(name="pt", bufs=3))       # 6
            oraw_pool = ctx.enter_context(tc.tile_pool(name="oraw", bufs=2))   # 20
            rcp_pool = ctx.enter_context(tc.tile_pool(name="rcp", bufs=1))     # 8
            rb_pool = ctx.enter_context(tc.tile_pool(name="rb", bufs=1))       # 16
            lmh_pool = ctx.enter_context(tc.tile_pool(name="lmh", bufs=1))     # 8
            lgout = ctx.enter_context(tc.tile_pool(name="lgout", bufs=2))      # 4
            lgb_pool = ctx.enter_context(tc.tile_pool(name="lgb", bufs=1))     # 8
            ps_acc = ctx.enter_context(tc.tile_pool(name="psacc", bufs=5, space="PSUM"))
            ps_st = ctx.enter_context(tc.tile_pool(name="psst", bufs=2, space="PSUM"))
            ps_ov = ctx.enter_context(tc.tile_pool(name="psov", bufs=1, space="PSUM"))
            dram = ctx.enter_context(tc.tile_pool(name="dram", bufs=1, space="DRAM"))

            ident = const.tile([128, 128], BF, name="ident")
            make_identity(nc, ident)
            eps_t = const.tile([128, 1], F32, name="eps")
            nc.vector.memset(eps_t[:], LN_EPS)
            maskT = const.tile([128, NCT, 128], BF, name="maskT")
            nc.sync.dma_start(out=maskT[:], in_=maskT_d[:])

            kv_loc = dram.tile([KV_SZ], BF, name="kv_loc")
            hfT_loc = dram.tile([C * TL], BF, name="hfT_loc")
            hfT_full = dram.tile([NCORES * C * TL], BF, addr_space="Local" if sim else "Shared", name="hfT_full")
            rc_bounce = dram.tile([SEQ, H * 128], F32, name="rc_bounce")

            # persistent residual stream, fp32: tile tt = seq tt, partition j
            x_t = [xpool.tile([128, C], F32, tag=f"x{tt}", name=f"x{tt}") for tt in range(NT)]
            for tt in range(NT):
                nc.sync.dma_start(out=x_t[tt][:], in_=x0[tt * 128:(tt + 1) * 128, :])

            def bcast_row(dst, src_1d_tensor, offset, n):
                # replicate a [n] dram row across all partitions of dst [P, n]
                src = _ap(src_1d_tensor, offset, [[0, dst.shape[0]], [1, n]])
                nc.gpsimd.dma_start(out=dst[:], in_=src)

            def emit_ln(g_t, b_t):
                """LayerNorm over free dim of x_t -> transposed bf16 hT tiles."""
                h_tiles = []
                for tt in range(NT):
                    stats = misc.tile([128, 2, 6], F32, name="stats", tag="stats")
                    xv = x_t[tt][:].rearrange("p (s d) -> p s d", s=2)
                    nc.vector.bn_stats(out=stats[:, 0, :], in_=xv[:, 0, :])
                    nc.vector.bn_stats(out=stats[:, 1, :], in_=xv[:, 1, :])
                    mv = misc.tile([128, 2], F32, name="mv", tag="mv")
                    nc.vector.bn_aggr(out=mv[:], in_=stats[:])
                    rstd = misc.tile([128, 1], F32, name="rstd", tag="rstd")
                    nc.scalar.activation(rstd[:], mv[:, 1:2], AF.Sqrt, bias=eps_t[:])
                    nc.vector.reciprocal(rstd[:], rstd[:])
                    xn = lntmp.tile([128, C], F32, tag="xn", name="xn")
                    nc.vector.tensor_scalar(
                        out=xn[:], in0=x_t[tt][:], scalar1=mv[:, 0:1], scalar2=rstd[:],
                        op0=mybir.AluOpType.subtract, op1=mybir.AluOpType.mult,
                    )
                    nc.vector.tensor_mul(out=xn[:], in0=xn[:], in1=g_t[:])
                    h = hpool.tile([128, C], BF, tag="h", name="h")
                    nc.vector.tensor_add(out=h[:], in0=xn[:], in1=b_t[:])
                    h_tiles.append(h)
                hT_tiles = []
                for ct in range(NCT):
                    pst = ps_st.tile([128, 512], BF, tag="st", name="pst")
                    for tt in range(NT):
                        nc.tensor.transpose(
                            pst[:, tt * 128:(tt + 1) * 128],
                            h_tiles[tt][:, ct * 128:(ct + 1) * 128],
                            ident[:],
                        )
                    hT = tpool.tile([128, 512], BF, tag=f"hT{ct}", name=f"hT{ct}")
                    nc.vector.tensor_copy(out=hT[:], in_=pst[:])
                    hT_tiles.append(hT)
                return hT_tiles

            def load_w_tile(wd, l_idx, r0, c0, rows=128, cols=512):
                wt = wpool.tile([rows, cols], BF, tag="w", name="wt")
                nc.sync.dma_start(out=wt[:], in_=wd[l_idx, r0:r0 + rows, c0:c0 + cols])
                return wt

            for l in range(LL):
                lw = l % L
                g1 = gbpool.tile([128, C], F32, tag="g", name="g1")
                bcast_row(g1, ln1g_d, lw * C, C)
                bb1 = gbpool.tile([128, C], F32, tag="b", name="bb1")
                bcast_row(bb1, ln1b_d, lw * C, C)
                hT = emit_ln(g1, bb1)

                # ---- QKV projections ----
                # Q^T (resident), K^T (streamed to kv_loc): out[d,t] = sum_c W[c,d] hT[c,t]
                qT = []
                for name, wd in (("q", wq_d), ("k", wk_d)):
                    for dtg in range(2):
                        pss = [ps_acc.tile([128, 512], F32, tag="acc", name="acc") for _ in range(4)]
                        for ct in range(NCT):
                            wt = load_w_tile(wd, lw, ct * 128, dtg * 512)
                            for d4 in range(4):
                                nc.tensor.matmul(
                                    pss[d4][:], wt[:, d4 * 128:(d4 + 1) * 128], hT[ct][:],
                                    start=(ct == 0), stop=(ct == NCT - 1),
                                )
                        for d4 in range(4):
                            dt = dtg * 4 + d4
                            if name == "q":
                                ot = qtpool.tile([128, 512], BF, tag=f"qT{dt}", name=f"qT{dt}")
                                nc.vector.tensor_copy(out=ot[:], in_=pss[d4][:])
                                qT.append(ot)
                            else:
                                ot = kvpool.tile([128, 512], BF, tag="kv", name="kTs_out")
                                nc.vector.tensor_copy(out=ot[:], in_=pss[d4][:])
                                nc.sync.dma_start(
                                    out=_ap(kv_loc, dt * 128 * TL, [[TL, 128], [1, TL]]),
                                    in_=ot[:],
                                )
                # V natural (streamed in halves): out[t,c'] = sum_c hT[c,t] Wv[c,c']
                for nf in range(2):
                    pss = [ps_acc.tile([128, 512], F32, tag="acc", name="acc") for _ in range(4)]
                    for ct in range(NCT):
                        wt = load_w_tile(wv_d, lw, ct * 128, nf * 512)
                        for tt in range(NT):
                            nc.tensor.matmul(
                                pss[tt][:], hT[ct][:, tt * 128:(tt + 1) * 128], wt[:],
                                start=(ct == 0), stop=(ct == NCT - 1),
                            )
                    for tt in range(NT):
                        vt = kvpool.tile([128, 512], BF, tag="kv", name="v_out")
                        nc.vector.tensor_copy(out=vt[:], in_=pss[tt][:])
                        nc.sync.dma_start(
                            out=_ap(kv_loc, KV_K + tt * 128 * C + nf * 512, [[C, 128], [1, 512]]),
                            in_=vt[:],
                        )
                kv_full = dram.tile([NCORES * KV_SZ], BF, addr_space="Local" if sim else "Shared", name=f"kv_full{l}")
                if sim:
                    nc.sync.dma_start(
                        out=_ap(kv_full, 0, [[2048, KV_SZ // 2048], [1, 2048]]),
                        in_=_ap(kv_loc, 0, [[2048, KV_SZ // 2048], [1, 2048]]),
                    )
                else:
                    nc.gpsimd.collective_compute(
                        "AllGather",
                        mybir.AluOpType.bypass,
                        replica_groups=[list(range(NCORES))],
                        ins=[_ap(kv_loc, 0, [[2048, KV_SZ // 2048], [1, 2048]])],
                        outs=[_ap(kv_full, 0, [[2048, NCORES * KV_SZ // 2048], [1, 2048]])],
                    )

                # ---- attention ----
                # O^T as one tile: row c = ct*128 + p, free = (ct, t)
                oT = otpool.tile([128, NCT, 512], BF, tag="oT", name="oT")
                for s in range(SEQ):
                    # V for seq s, all ranks/heads, with a ones column per head:
                    # v_s[j, r, h, 0:64] = V_r[s*128+j, h*64+d]; v_s[..., 64] = 1
                    v_s = big.tile([128, NCT, H, HD + 1], BF, tag="big", name="vs")
                    nc.vector.memset(v_s[:, :, :, HD:HD + 1], 1.0)
                    for r in range(NCORES):
                        nc.sync.dma_start(
                            out=v_s[:, r, :, 0:HD],
                            in_=_ap(
                                kv_full,
                                r * KV_SZ + KV_K + s * 128 * C,
                                [[C, 128], [HD, H], [1, HD]],
                            ),
                        )
                    oraw = oraw_pool.tile([HD + 1, H, 128], F32, tag="oraw", name="oraw")
                    for h in range(H):
                        poff = (h % 2) * HD  # parity offset matches qT slices
                        kTs = kts_pool.tile([128, NCT, 128], BF, tag="kts", name="kts")
                        nc.sync.dma_start(
                            out=kTs[poff:poff + HD, :, :],
                            in_=_ap(
                                kv_full,
                                h * HD * TL + s * 128,
                                [[TL, HD], [KV_SZ, NCORES], [1, 128]],
                            ),
                        )
                        q_sl = qT[h // 2][poff:poff + HD, s * 128:(s + 1) * 128]
                        pT = pt_pool.tile([128, NCT, 128], BF, tag="pt", name="pt")
                        for half in range(2):
                            st = ps_st.tile([128, 4, 128], F32, tag="st", name="st")
                            for k4 in range(4):
                                nc.tensor.matmul(
                                    st[:, k4, :], kTs[poff:poff + HD, half * 4 + k4, :], q_sl,
                                    start=True, stop=True,
                                )
                            nc.scalar.activation(
                                pT[:, half * 4:half * 4 + 4, :], st[:], AF.Exp
                            )
                        nc.vector.tensor_mul(out=pT[:], in0=pT[:], in1=maskT[:])
                        ov = ps_ov.tile([128, 128], F32, tag="ov", name="ov")
                        for kt in range(NCT):
                            nc.tensor.matmul(
                                ov[0:HD + 1, :], v_s[:, kt, h, :], pT[:, kt, :],
                                start=(kt == 0), stop=(kt == NCT - 1),
                            )
                        nc.vector.tensor_copy(out=oraw[:, h, :], in_=ov[0:HD + 1, :])
                    # denominators -> reciprocal -> broadcast over 64 partitions
                    recips = rcp_pool.tile([1, H, 128], F32, tag="recips", name="recips")
                    nc.vector.reciprocal(recips[:], oraw[HD:HD + 1, :, :])
                    nc.sync.dma_start(out=rc_bounce[s, :], in_=recips[:])
                    rb = rb_pool.tile([HD, H, 128], F32, tag="rb", name="rb")
                    nc.gpsimd.dma_start(
                        out=rb[:], in_=_ap(rc_bounce, s * H * 128, [[0, HD], [128, H], [1, 128]])
                    )
                    # even heads: normalize straight into oT (partitions 0-63);
                    # odd heads: stage then DMA into partitions 64-127
                    oS = oraw_pool.tile([HD, NCT, 128], BF, tag="oS", name="oS")
                    for h in range(H):
                        if h % 2 == 0:
                            dst = oT[0:HD, h // 2, s * 128:(s + 1) * 128]
                        else:
                            dst = oS[:, h // 2, :]
                        nc.vector.tensor_mul(out=dst, in0=oraw[0:HD, h, :], in1=rb[:, h, :])
                    nc.sync.dma_start(
                        out=oT[HD:128, :, s * 128:(s + 1) * 128], in_=oS[:],
                    )

                # ---- output projection + residual ----
                bo_t = gbpool.tile([128, C], F32, tag="b", name="bo_t")
                bcast_row(bo_t, bo_d, lw * C, C)
                for nf in range(2):
                    pss = [ps_acc.tile([128, 512], F32, tag="acc", name="acc") for _ in range(4)]
                    for ct in range(NCT):
                        wt = load_w_tile(wo_d, lw, ct * 128, nf * 512)
                        for tt in range(NT):
                            nc.tensor.matmul(
                                pss[tt][:], oT[:, ct, tt * 128:(tt + 1) * 128], wt[:],
                                start=(ct == 0), stop=(ct == NCT - 1),
                            )
                    for tt in range(NT):
                        xs = x_t[tt][:, nf * 512:(nf + 1) * 512]
                        nc.vector.tensor_add(out=xs, in0=xs, in1=pss[tt][:])
                        nc.vector.tensor_add(out=xs, in0=xs, in1=bo_t[:, nf * 512:(nf + 1) * 512])

                # ---- FFN ----
                g2 = gbpool.tile([128, C], F32, tag="g", name="g2")
                bcast_row(g2, ln2g_d, lw * C, C)
                bb2 = gbpool.tile([128, C], F32, tag="b", name="bb2")
                bcast_row(bb2, ln2b_d, lw * C, C)
                h2T = emit_ln(g2, bb2)

                b1_t = misc.tile([128, NFT], F32, tag="b1", name="b1_t")
                nc.gpsimd.dma_start(
                    out=b1_t[:], in_=_ap(b1_d, lw * FF, [[1, 128], [128, NFT]])
                )
                ug = big.tile([128, NFT, 512], BF, tag="big", name="ug")
                for fg in range(8):
                    pss = [ps_acc.tile([128, 512], F32, tag="acc", name="acc") for _ in range(4)]
                    for ct in range(NCT):
                        wt = load_w_tile(w1_d, lw, ct * 128, fg * 512)
                        for f4 in range(4):
                            nc.tensor.matmul(
                                pss[f4][:], wt[:, f4 * 128:(f4 + 1) * 128], h2T[ct][:],
                                start=(ct == 0), stop=(ct == NCT - 1),
                            )
                    for f4 in range(4):
                        ft = fg * 4 + f4
                        nc.scalar.activation(
                            ug[:, ft, :], pss[f4][:], AF.Gelu, bias=b1_t[:, ft:ft + 1]
                        )

                b2_t = gbpool.tile([128, C], F32, tag="b", name="b2_t")
                bcast_row(b2_t, b2_d, lw * C, C)
                for nf in range(2):
                    pss = [ps_acc.tile([128, 512], F32, tag="acc", name="acc") for _ in range(4)]
                    for ft in range(NFT):
                        wt = load_w_tile(w2_d, lw, ft * 128, nf * 512)
                        for tt in range(NT):
                            nc.tensor.matmul(
                                pss[tt][:], ug[:, ft, tt * 128:(tt + 1) * 128], wt[:],
                                start=(ft == 0), stop=(ft == NFT - 1),
                            )
                    for tt in range(NT):
                        xs = x_t[tt][:, nf * 512:(nf + 1) * 512]
                        nc.vector.tensor_add(out=xs, in0=xs, in1=pss[tt][:])
                        nc.vector.tensor_add(out=xs, in0=xs, in1=b2_t[:, nf * 512:(nf + 1) * 512])

                if debug:
                    for tt in range(NT):
                        nc.sync.dma_start(
                            out=dbg_d[l, tt * 128:(tt + 1) * 128, :], in_=x_t[tt][:]
                        )

            # ---- final LN, AllGather h_f^T, lm_head ----
            gf = gbpool.tile([128, C], F32, tag="g", name="gf")
            bcast_row(gf, lnfg_d, 0, C)
            bft = gbpool.tile([128, C], F32, tag="b", name="bft")
            bcast_row(bft, lnfb_d, 0, C)
            hfT = emit_ln(gf, bft)
            for ct in range(NCT):
                nc.sync.dma_start(
                    out=_ap(hfT_loc, ct * 128 * TL, [[TL, 128], [1, TL]]),
                    in_=hfT[ct][:],
                )
            if sim:
                nc.sync.dma_start(
                    out=_ap(hfT_full, 0, [[2048, C * TL // 2048], [1, 2048]]),
                    in_=_ap(hfT_loc, 0, [[2048, C * TL // 2048], [1, 2048]]),
                )
            else:
                nc.gpsimd.collective_compute(
                    "AllGather",
                    mybir.AluOpType.bypass,
                    replica_groups=[list(range(NCORES))],
                    ins=[_ap(hfT_loc, 0, [[2048, C * TL // 2048], [1, 2048]])],
                    outs=[_ap(hfT_full, 0, [[2048, NCORES * C * TL // 2048], [1, 2048]])],
                )

            # lm_head in vocab halves: wlm half resident, hf streamed per r
            for vq in range(2):
                wlm_q = big.tile([128, NCT, 4 * VCW], BF, tag="big", name="wlmq")
                for ct in range(NCT):
                    nc.sync.dma_start(
                        out=wlm_q[:, ct, :],
                        in_=wlm_d[ct * 128:(ct + 1) * 128, vq * 4 * VCW:(vq + 1) * 4 * VCW],
                    )
                blm_qs = []
                for v4 in range(4):
                    bq = lgb_pool.tile([128, VCW], BF, tag="lgb", name="blmq", bufs=4)
                    bcast_row(bq, blm_d, (vq * 4 + v4) * VCW, VCW)
                    blm_qs.append(bq)
                for r in range(NCORES):
                    hfr = lmh_pool.tile([128, NCT, 512], BF, tag="hfr", name="hfr")
                    for ct in range(NCT):
                        nc.sync.dma_start(
                            out=hfr[:, ct, :],
                            in_=_ap(hfT_full, r * C * TL + ct * 128 * TL, [[TL, 128], [1, TL]]),
                        )
                    for ts in range(NT):
                        pss = [ps_acc.tile([128, VCW], F32, tag="acc", name="acc") for _ in range(4)]
                        for ct in range(NCT):
                            for v4 in range(4):
                                nc.tensor.matmul(
                                    pss[v4][:],
                                    hfr[:, ct, ts * 128:(ts + 1) * 128],
                                    wlm_q[:, ct, v4 * VCW:(v4 + 1) * VCW],
                                    start=(ct == 0), stop=(ct == NCT - 1),
                                )
                        for v4 in range(4):
                            vc = vq * 4 + v4
                            lg = lgout.tile([128, VCW], F32, tag="lg", name="lg")
                            nc.vector.tensor_add(
                                out=lg[:], in0=pss[v4][:],
                                in1=blm_qs[v4][:],
                            )
                            row0 = r * TL + ts * 128
                            nc.sync.dma_start(
                                out=logits_d[row0:row0 + 128, vc * VCW:(vc + 1) * VCW],
                                in_=lg[:],
                            )

    nc.compile()
    _prog_cache[key] = nc
    return nc


def _prep_inputs(inputs):
    f = {k: np.asarray(v) for k, v in inputs.items()}
    idx = f["idx"].astype(np.int64)
    emb = f["emb"].astype(np.float32)
    pos = f["pos_enc"].astype(np.float32)
    x_full = emb[idx] + pos[None, :, :]          # [B, T, C] f32

    scale = HD ** -0.5
    bf = lambda a: np.ascontiguousarray(a, dtype=np.float32).astype(BF16NP)
    shared = {
        "wq": bf(f["Wq"] * scale),
        "wk": bf(f["Wk"]),
        "wv": bf(f["Wv"]),
        "wo": bf(f["Wo"]),
        "w1": bf(f["W1"]),
        "w2": bf(f["W2"]),
        "b1": f["b1"].astype(np.float32),
        "bo": f["bo"].astype(np.float32),
        "b2": f["b2"].astype(np.float32),
        "ln1g": f["ln1_g"].astype(np.float32),
        "ln1b": f["ln1_b"].astype(np.float32),
        "ln2g": f["ln2_g"].astype(np.float32),
        "ln2b": f["ln2_b"].astype(np.float32),
        "lnfg": f["lnf_g"].astype(np.float32),
        "lnfb": f["lnf_b"].astype(np.float32),
    }
    wlm_f = f["Wlm"].astype(np.float32)
    blm_f = f["blm"].astype(np.float32)

    in_maps = []
    kk = np.arange(T)[:, None]
    for c in range(NCORES):
        x0_c = np.ascontiguousarray(
            x_full[:, 128 * c:128 * (c + 1), :].reshape(TL, C), dtype=np.float32
        )
        jj = np.arange(128)[None, :]
        m = (kk <= 128 * c + jj).astype(np.float32)      # [T, 128]
        maskT_c = np.ascontiguousarray(
            m.reshape(NCT, 128, 128).transpose(1, 0, 2)
        ).astype(BF16NP)                                  # [128(kk), 8(kt), 128(j)]
        im = dict(shared)
        im["x0"] = x0_c
        im["maskT"] = maskT_c
        im["wlm"] = np.ascontiguousarray(wlm_f[:, c * VSH:(c + 1) * VSH]).astype(BF16NP)
        im["blm"] = np.ascontiguousarray(blm_f[c * VSH:(c + 1) * VSH]).astype(BF16NP)
        in_maps.append(im)
    return in_maps


def kernel(**inputs):
    nc = _build()
    in_maps = _prep_inputs(inputs)
    res = run_bass_kernel_spmd(nc, in_maps, list(range(NCORES)))
    # per-core logits rows are [r(8), s(4), j(128)]; vocab sharded on cores
    parts = [r["logits"].reshape(NCORES, SEQ, 128, VSH) for r in res.results]
    full = np.concatenate(parts, axis=-1)                 # [r, s, j, V]
    full = full.transpose(1, 0, 2, 3).reshape(B, T, V)    # [s, r*128+j, V]
    return np.ascontiguousarray(full, dtype=np.float32)



# revision 23
# speedup vs baseline: 1.1549x; 1.1549x over previous
"""GPT forward pass on 8 Trainium2 NeuronCores — v2 (head-sharded attention).

Trunk (LN/FFN/residual/lm_head) is token-parallel: core c owns token tile c
(128 tokens) of each of the 4 sequences. Attention is Megatron head-sharded:
core c owns global heads {2c, 2c+1} and computes full causal attention for
those heads over all 4096 tokens — every (head, seq) unit has the identical
causal block structure (q-tile qt needs qt+1 k-tiles), so the SPMD program
is uniform across cores while skipping all fully-masked blocks (36/64).

Per layer: LN1 -> AllGather h^T (1 MB payload) -> Q/K/V for my 2 heads over
all tokens (SBUF-resident, no DRAM round-trip) -> causal attention ->
row-parallel Wo partial -> bf16 ReduceScatter back to token owners ->
residual -> LN2 -> FFN (token-parallel). lm_head is token-sharded: each
core streams the full lnf-folded Wlm and emits bf16 logits for its 512
tokens; blm is added on the host.

LN gains are folded into consuming weights host-side (exact); ln2/lnf
biases into b1/blm (exact); projection bias b2 enters the PSUM via a K=1
ones-row matmul; bo is added after the ReduceScatter. All matmuls bf16
with fp32 PSUM; residual stream and softmax stats stay fp32. Softmax is
transposed-layout with denominators from a ones-column of V (no max
subtraction needed at these scales — matches the reference to ~5e-3).
"""

import os
import sys

for _p in ("/opt/trn_rl_repo",):
    if os.path.isdir(_p) and _p not in sys.path:
        sys.path.insert(0, _p)

import numpy as np
import ml_dtypes

BF16NP = ml_dtypes.bfloat16

import concourse.bass as bass
import concourse.mybir as mybir
import concourse.tile as tile
from concourse import bacc
from concourse.bass_utils import run_bass_kernel_spmd
from concourse.masks import make_identity

F32 = mybir.dt.float32
BF = mybir.dt.bfloat16
AF = mybir.ActivationFunctionType

V, C, T, H, L, B = 32000, 1024, 1024, 16, 4, 4
HD = C // H          # 64
FF = 4 * C           # 4096
NCORES = 8
TL = 512             # local tokens per core (4 seqs x 128)
TT = B * T // 1      # 4096 total tokens (seq-major: t = s*1024 + pos)
SEQ = B
NT = TL // 128       # 4 local t-tiles; tile tt = seq tt
NCT = C // 128       # 8 c-tiles
NFT = FF // 128      # 32 f-tiles
NTB = TT // 128      # 32 global t-blocks
NCH = 64             # vocab chunks
VCW = V // NCH       # 500
LN_EPS = 1e-5

_prog_cache = {}


def _ap(t, offset, pattern):
    return bass.AP(tensor=t.tensor if isinstance(t, bass.AP) else t, offset=offset, ap=pattern)


def _build(LL=L, sim=False):
    key = (LL, sim)
    if key in _prog_cache:
        return _prog_cache[key]

    nc = bacc.Bacc("TRN2", target_bir_lowering=False, debug=False, num_devices=NCORES)

    x0 = nc.dram_tensor("x0", [TL, C], F32, kind="ExternalInput")
    mask_d = nc.dram_tensor("maskd", [128, 128], BF, kind="ExternalInput")
    wq_d = nc.dram_tensor("wq", [L, C, 128], BF, kind="ExternalInput")   # my head cols
    wk_d = nc.dram_tensor("wk", [L, C, 128], BF, kind="ExternalInput")
    wv_d = nc.dram_tensor("wv", [L, C, 128], BF, kind="ExternalInput")
    wo_d = nc.dram_tensor("wo", [L, 128, C], BF, kind="ExternalInput")   # my head rows
    w1_d = nc.dram_tensor("w1", [L, C, FF], BF, kind="ExternalInput")
    w2_d = nc.dram_tensor("w2", [L, FF, C], BF, kind="ExternalInput")
    b1_d = nc.dram_tensor("b1", [L, FF], F32, kind="ExternalInput")
    bo_d = nc.dram_tensor("bo", [L, C], F32, kind="ExternalInput")
    b2_d = nc.dram_tensor("b2", [L, C], BF, kind="ExternalInput")
    wlm_d = nc.dram_tensor("wlm", [NCH, NCT, 128, VCW], BF, kind="ExternalInput")

    logits_d = nc.dram_tensor("logits", [TL, V], BF, kind="ExternalOutput")

    HTSZ = C * TL            # elems in one core's hT payload (1 MB bf16)

    with tile.TileContext(nc) as tc:
        import contextlib

        with contextlib.ExitStack() as ctx:
            # SBUF pools (~per-partition KB)
            const = ctx.enter_context(tc.tile_pool(name="const", bufs=1))      # .6
            xpool = ctx.enter_context(tc.tile_pool(name="x", bufs=1))          # 16
            hpool = ctx.enter_context(tc.tile_pool(name="h", bufs=5))          # 10
            tpool = ctx.enter_context(tc.tile_pool(name="hT", bufs=1))         # 8
            big = ctx.enter_context(tc.tile_pool(name="big", bufs=2))          # 64
            qkt = ctx.enter_context(tc.tile_pool(name="qkt", bufs=1))          # 16
            vsb_p = ctx.enter_context(tc.tile_pool(name="vsb", bufs=1))        # 8.3
            otm_p = ctx.enter_context(tc.tile_pool(name="otm", bufs=1))        # 8
            w4 = ctx.enter_context(tc.tile_pool(name="w4", bufs=4))            # 16
            qwp = ctx.enter_context(tc.tile_pool(name="qw", bufs=1))           # 6
            gbpool = ctx.enter_context(tc.tile_pool(name="gb", bufs=1))        # 4.5
            misc = ctx.enter_context(tc.tile_pool(name="misc", bufs=2))        # .6
            pt_pool = ctx.enter_context(tc.tile_pool(name="pt", bufs=3))       # 6
            oraw_pool = ctx.enter_context(tc.tile_pool(name="oraw", bufs=2))   # 8
            os_pool = ctx.enter_context(tc.tile_pool(name="oS", bufs=2))       # 4
            rcp_pool = ctx.enter_context(tc.tile_pool(name="rcp", bufs=2))     # 8
            rb_pool = ctx.enter_context(tc.tile_pool(name="rb", bufs=2))       # 8
            pd_pool = ctx.enter_context(tc.tile_pool(name="pd", bufs=3))       # 6
            rcv_pool = ctx.enter_context(tc.tile_pool(name="rcv", bufs=2))     # 4
            lgout = ctx.enter_context(tc.tile_pool(name="lgout", bufs=2))      # 8
            ps_acc = ctx.enter_context(tc.tile_pool(name="psacc", bufs=4, space="PSUM"))
            ps_st = ctx.enter_context(tc.tile_pool(name="psst", bufs=2, space="PSUM"))
            ps_ov = ctx.enter_context(tc.tile_pool(name="psov", bufs=2, space="PSUM"))
            dram = ctx.enter_context(tc.tile_pool(name="dram", bufs=1, space="DRAM"))

            ident = const.tile([128, 128], BF, name="ident")
            make_identity(nc, ident)
            eps_t = const.tile([128, 1], F32, name="eps")
            nc.vector.memset(eps_t[:], LN_EPS)
            mask_t = const.tile([128, 128], BF, name="mask")
            nc.sync.dma_start(out=mask_t[:], in_=mask_d[:])
            ones1 = const.tile([1, 128], BF, name="ones1")
            nc.vector.memset(ones1[:], 1.0)

            hT_loc = dram.tile([HTSZ], BF, name="hT_loc")
            part_loc = dram.tile([TT * C], BF, name="part_loc")

            # persistent residual stream fp32: tile tt = seq tt
            x_t = [xpool.tile([128, C], F32, tag=f"x{tt}", name=f"x{tt}") for tt in range(NT)]
            for tt in range(NT):
                nc.sync.dma_start(out=x_t[tt][:], in_=x0[tt * 128:(tt + 1) * 128, :])

            def emit_ln():
                """x_t -> (x-m)*rstd bf16, transposed hT tiles (g/b folded away)."""
                h_tiles = []
                for tt in range(NT):
                    stats = misc.tile([128, 2, 6], F32, name="stats", tag="stats")
                    xv = x_t[tt][:].rearrange("p (s d) -> p s d", s=2)
                    nc.vector.bn_stats(out=stats[:, 0, :], in_=xv[:, 0, :])
                    nc.vector.bn_stats(out=stats[:, 1, :], in_=xv[:, 1, :])
                    mv = misc.tile([128, 2], F32, name="mv", tag="mv")
                    nc.vector.bn_aggr(out=mv[:], in_=stats[:])
                    rstd = misc.tile([128, 1], F32, name="rstd", tag="rstd")
                    nc.scalar.activation(rstd[:], mv[:, 1:2], AF.Sqrt, bias=eps_t[:])
                    nc.vector.reciprocal(rstd[:], rstd[:])
                    h = hpool.tile([128, C], BF, tag="h", name="h")
                    nc.vector.tensor_scalar(
                        out=h[:], in0=x_t[tt][:], scalar1=mv[:, 0:1], scalar2=rstd[:],
                        op0=mybir.AluOpType.subtract, op1=mybir.AluOpType.mult,
                    )
                    h_tiles.append(h)
                hT_all = tpool.tile([128, NCT, 512], BF, tag="hTall", name="hTall")
                for ct in range(NCT):
                    pst = ps_st.tile([128, 512], BF, tag="st", name="pst")
                    for tt in range(NT):
                        nc.tensor.transpose(
                            pst[:, tt * 128:(tt + 1) * 128],
                            h_tiles[tt][:, ct * 128:(ct + 1) * 128],
                            ident[:],
                        )
                    nc.vector.tensor_copy(out=hT_all[:, ct, :], in_=pst[:])
                return hT_all

            for l in range(LL):
                lw = l % L
                # per-layer bias/const tiles (issued early; Pool queue quiet)
                b2_r = gbpool.tile([1, C], BF, tag="b2r", name="b2_r")
                nc.sync.dma_start(out=b2_r[:], in_=b2_d[lw:lw + 1, :])
                bo_b = gbpool.tile([128, C], F32, tag="bob", name="bo_b")
                nc.gpsimd.dma_start(
                    out=bo_b[:], in_=_ap(bo_d, lw * C, [[0, 128], [1, C]])
                )
                b1_t = misc.tile([128, NFT], F32, tag="b1", name="b1_t")
                nc.gpsimd.dma_start(
                    out=b1_t[:], in_=_ap(b1_d, lw * FF, [[1, 128], [128, NFT]])
                )
                # my-head projection weights [128c-in-ct, ct, 128d] — one DMA each
                wq_t = qwp.tile([128, NCT, 128], BF, tag="qw", name="wq_t")
                wk_t = qwp.tile([128, NCT, 128], BF, tag="kw", name="wk_t")
                wv_t = qwp.tile([128, NCT, 128], BF, tag="vw", name="wv_t")
                for wt, wd in ((wq_t, wq_d), (wk_t, wk_d), (wv_t, wv_d)):
                    nc.sync.dma_start(
                        out=wt[:],
                        in_=_ap(wd[0:1, 0:1, 0:1], lw * C * 128,
                                [[128, 128], [128 * 128, NCT], [1, 128]]),
                    )
                wo_t = w4.tile([128, 2, 512], BF, tag="w", name="wo_t")
                nc.sync.dma_start(
                    out=wo_t[:],
                    in_=_ap(wo_d[0:1, 0:1, 0:1], lw * 128 * C, [[C, 128], [512, 2], [1, 512]]),
                )

                # ---- LN1 -> hT, publish (one DMA), AllGather ----
                hT = emit_ln()
                nc.sync.dma_start(
                    out=_ap(hT_loc, 0, [[512, 128], [128 * 512, NCT], [1, 512]]),
                    in_=hT[:],
                )
                hT_full = dram.tile(
                    [NCORES * HTSZ], BF,
                    addr_space="Local" if sim else "Shared", name=f"hT_full{l}",
                )
                if sim:
                    nc.sync.dma_start(
                        out=_ap(hT_full, 0, [[2048, HTSZ // 2048], [1, 2048]]),
                        in_=_ap(hT_loc, 0, [[2048, HTSZ // 2048], [1, 2048]]),
                    )
                else:
                    nc.gpsimd.collective_compute(
                        "AllGather",
                        mybir.AluOpType.bypass,
                        replica_groups=[list(range(NCORES))],
                        ins=[_ap(hT_loc, 0, [[2048, HTSZ // 2048], [1, 2048]])],
                        outs=[_ap(hT_full, 0, [[2048, NCORES * HTSZ // 2048], [1, 2048]])],
                    )

                # gathered h^T lands per rank-chunk so QKV(seq s) starts after
                # 2 chunks, and attention(s) overlaps QKV(s+1)
                hT_sb = [None, None]

                def load_chunk(ch):
                    half = ch // 4
                    if hT_sb[half] is None:
                        hT_sb[half] = big.tile(
                            [128, 4, NCT, 512], BF, tag="big", name=f"hTsb{half}"
                        )
                    nc.sync.dma_start(
                        out=hT_sb[half][:, ch % 4, :, :],
                        in_=_ap(hT_full, ch * HTSZ, [[512, 128], [128 * 512, NCT], [1, 512]]),
                    )

                def htf(ct, ch):
                    return hT_sb[ch // 4][:, ch % 4, ct, :]

                qT_s, kT_s, v_ss, oTm_s = [], [], [], []
                for s in range(SEQ):
                    qT_s.append(qkt.tile([128, 1024], BF, tag=f"qT{s}", name=f"qT{s}"))
                    kT_s.append(qkt.tile([128, 1024], BF, tag=f"kT{s}", name=f"kT{s}"))
                    v_ss.append(vsb_p.tile([128, 8, 2, HD + 1], BF, tag=f"v{s}", name=f"v{s}"))
                    oTm_s.append(otm_p.tile([128, 1024], BF, tag=f"oTm{s}", name=f"oTm{s}"))

                for s in range(SEQ):
                    for hh in range(2):
                        load_chunk(2 * s + hh)
                    # K^T then Q^T for this seq (2 chunks each)
                    for dst, wt_l in ((kT_s[s], wk_t), (qT_s[s], wq_t)):
                        for hh in range(2):
                            ch = 2 * s + hh
                            ps = ps_acc.tile([128, 512], F32, tag="acc", name="acc")
                            for ct in range(NCT):
                                nc.tensor.matmul(
                                    ps[:], wt_l[:, ct, :], htf(ct, ch),
                                    start=(ct == 0), stop=(ct == NCT - 1),
                                )
                            nc.vector.tensor_copy(
                                out=dst[:, hh * 512:(hh + 1) * 512], in_=ps[:]
                            )
                    # V natural [t, my 128 d] with ones column
                    v_sb = v_ss[s]
                    nc.vector.memset(v_sb[:, :, :, HD:HD + 1], 1.0)
                    for i in range(8):
                        tb = s * 8 + i
                        ps = ps_acc.tile([128, 128], F32, tag="acc", name="psv")
                        for ct in range(NCT):
                            nc.tensor.matmul(
                                ps[:], htf(ct, tb // 4)[:, (tb % 4) * 128:(tb % 4 + 1) * 128],
                                wv_t[:, ct, :],
                                start=(ct == 0), stop=(ct == NCT - 1),
                            )
                        nc.vector.tensor_copy(out=v_sb[:, i, 0, 0:HD], in_=ps[:, 0:HD])
                        nc.vector.tensor_copy(out=v_sb[:, i, 1, 0:HD], in_=ps[:, HD:2 * HD])

                    # ---- causal attention for this seq, both heads ----
                    oraw = [
                        oraw_pool.tile([HD + 1, 1024], F32, tag="oraw", name="oraw")
                        for _ in range(2)
                    ]
                    for qt in range(8):
                        kept = qt + 1
                        for hp in range(2):
                            poff = hp * HD
                            q_sl = qT_s[s][poff:poff + HD, qt * 128:(qt + 1) * 128]
                            pT = pt_pool.tile([128, 8, 128], BF, tag="pt", name="pt")
                            for half in range((kept + 3) // 4):
                                cnt = min(4, kept - half * 4)
                                st = ps_st.tile([128, 4, 128], F32, tag="st", name="st")
                                for k4 in range(cnt):
                                    kt = half * 4 + k4
                                    nc.tensor.matmul(
                                        st[:, k4, :],
                                        kT_s[s][poff:poff + HD, kt * 128:(kt + 1) * 128],
                                        q_sl, start=True, stop=True,
                                    )
                                nc.scalar.activation(
                                    pT[:, half * 4:half * 4 + cnt, :], st[:, 0:cnt, :], AF.Exp
                                )
                            nc.vector.tensor_mul(
                                out=pT[:, qt, :], in0=pT[:, qt, :], in1=mask_t[:]
                            )
                            ov = ps_ov.tile([128, 128], F32, tag="ov", name="ov")
                            for i in range(kept):
                                nc.tensor.matmul(
                                    ov[0:HD + 1, :], v_sb[:, i, hp, :], pT[:, i, :],
                                    start=(i == 0), stop=(i == kept - 1),
                                )
                            nc.vector.tensor_copy(
                                out=oraw[hp][:, qt * 128:(qt + 1) * 128], in_=ov[0:HD + 1, :]
                            )
                    for hp in range(2):
                        recips = rcp_pool.tile([1, 1024], F32, tag="recips", name="recips")
                        nc.vector.reciprocal(recips[:], oraw[hp][HD:HD + 1, :])
                        rc_b = dram.tile([1024], F32, name=f"rcb{l}_{s}_{hp}")
                        nc.sync.dma_start(out=rc_b[:], in_=recips[:])
                        rb = rb_pool.tile([HD, 1024], F32, tag="rb", name="rb")
                        nc.gpsimd.dma_start(out=rb[:], in_=_ap(rc_b, 0, [[0, HD], [1, 1024]]))
                        if hp == 0:
                            nc.gpsimd.tensor_mul(
                                out=oTm_s[s][0:HD, :], in0=oraw[hp][0:HD, :], in1=rb[:]
                            )
                        else:
                            oS = os_pool.tile([HD, 1024], BF, tag="oS", name="oS")
                            nc.gpsimd.tensor_mul(out=oS[:], in0=oraw[hp][0:HD, :], in1=rb[:])
                            nc.sync.dma_start(out=oTm_s[s][HD:128, :], in_=oS[:])

                    # Wo partials for this seq fill PE while the next seq's
                    # exp/mask chains run; drains on ACT (2 table swaps/seq)
                    for p in range(8):
                        row0 = s * 1024 + p * 128  # flat token row; RS shard c = core c's tokens
                        pd = pd_pool.tile([128, C], BF, tag="pd", name="pd")
                        for cc in range(2):
                            ps = ps_acc.tile([128, 512], F32, tag="acc", name="acc")
                            nc.tensor.matmul(
                                ps[:], oTm_s[s][:, p * 128:(p + 1) * 128], wo_t[:, cc, :],
                                start=True, stop=True,
                            )
                            nc.scalar.activation(pd[:, cc * 512:(cc + 1) * 512], ps[:], AF.Copy)
                        nc.sync.dma_start(
                            out=_ap(part_loc, row0 * C, [[C, 128], [1, C]]),
                            in_=pd[:],
                        )
                recv = dram.tile([TL * C], BF, name=f"recv{l}")
                if sim:
                    nc.sync.dma_start(
                        out=_ap(recv, 0, [[2048, TL * C // 2048], [1, 2048]]),
                        in_=_ap(part_loc, 0, [[2048, TL * C // 2048], [1, 2048]]),
                    )
                else:
                    nc.gpsimd.collective_compute(
                        "ReduceScatter",
                        mybir.AluOpType.add,
                        replica_groups=[list(range(NCORES))],
                        ins=[_ap(part_loc, 0, [[2048, TT * C // 2048], [1, 2048]])],
                        outs=[_ap(recv, 0, [[2048, TL * C // 2048], [1, 2048]])],
                    )
                for tt in range(NT):
                    rt = rcv_pool.tile([128, C], BF, tag="rcv", name="rt")
                    nc.sync.dma_start(
                        out=rt[:], in_=_ap(recv, tt * 128 * C, [[C, 128], [1, C]])
                    )
                    nc.vector.tensor_add(out=x_t[tt][:], in0=x_t[tt][:], in1=rt[:])
                    nc.vector.tensor_add(out=x_t[tt][:], in0=x_t[tt][:], in1=bo_b[:])

                # ---- FFN (token-parallel, biases in-psum / in-activation) ----
                h2T = emit_ln()
                ug = big.tile([128, NFT, 512], BF, tag="big", name="ug")
                for fg in range(8):
                    pss = [ps_acc.tile([128, 512], F32, tag="acc", name="acc") for _ in range(4)]
                    for hf in range(2):
                        wt = w4.tile([128, 4, 512], BF, tag="w", name="w1t")
                        nc.sync.dma_start(
                            out=wt[:],
                            in_=_ap(w1_d[0:1, 0:1, 0:1],
                                    lw * C * FF + hf * 512 * FF + fg * 512,
                                    [[FF, 128], [128 * FF, 4], [1, 512]]),
                        )
                        for ci in range(4):
                            ct = hf * 4 + ci
                            for f4 in range(4):
                                nc.tensor.matmul(
                                    pss[f4][:], wt[:, ci, f4 * 128:(f4 + 1) * 128], h2T[:, ct, :],
                                    start=(ct == 0), stop=(ct == NCT - 1),
                                )
                    for f4 in range(4):
                        ft = fg * 4 + f4
                        nc.scalar.activation(
                            ug[:, ft, :], pss[f4][:], AF.Gelu, bias=b1_t[:, ft:ft + 1]
                        )
                for nf in range(2):
                    pss = [ps_acc.tile([128, 512], F32, tag="acc", name="acc") for _ in range(4)]
                    for g8 in range(8):
                        wt = w4.tile([128, 4, 512], BF, tag="w", name="w2t")
                        nc.sync.dma_start(
                            out=wt[:],
                            in_=_ap(w2_d[0:1, 0:1, 0:1],
                                    lw * FF * C + g8 * 512 * C + nf * 512,
                                    [[C, 128], [128 * C, 4], [1, 512]]),
                        )
                        for fi in range(4):
                            ft = g8 * 4 + fi
                            for tt in range(NT):
                                nc.tensor.matmul(
                                    pss[tt][:], ug[:, ft, tt * 128:(tt + 1) * 128], wt[:, fi, :],
                                    start=(ft == 0), stop=False,
                                )
                    for tt in range(NT):
                        nc.tensor.matmul(
                            pss[tt][:], ones1[:], b2_r[:, nf * 512:(nf + 1) * 512],
                            start=False, stop=True,
                        )
                        xs = x_t[tt][:, nf * 512:(nf + 1) * 512]
                        nc.vector.tensor_add(out=xs, in0=xs, in1=pss[tt][:])

            # ---- final LN (folded) + token-sharded lm_head ----
            hfT = emit_ln()
            for ch in range(NCH):
                wlm_c = big.tile([128, NCT, VCW], BF, tag="big", name="wlm_c")
                nc.sync.dma_start(
                    out=wlm_c[:],
                    in_=_ap(wlm_d[0:1, 0:1, 0:1, 0:1], ch * NCT * 128 * VCW,
                            [[VCW, 128], [128 * VCW, NCT], [1, VCW]]),
                )
                for th in range(2):
                    lg = lgout.tile([128, 2, VCW], BF, tag="lg", name="lg")
                    for ti in range(2):
                        tt = th * 2 + ti
                        ps = ps_acc.tile([128, VCW], F32, tag="acc", name="acc")
                        for ct in range(NCT):
                            nc.tensor.matmul(
                                ps[:], hfT[:, ct, tt * 128:(tt + 1) * 128], wlm_c[:, ct, :],
                                start=(ct == 0), stop=(ct == NCT - 1),
                            )
                        if ti % 2 == 0:
                            nc.vector.tensor_copy(out=lg[:, ti, :], in_=ps[:])
                        else:
                            nc.scalar.activation(lg[:, ti, :], ps[:], AF.Copy)
                    nc.sync.dma_start(
                        out=_ap(logits_d[0:1, 0:1], th * 2 * 128 * V + ch * VCW,
                                [[V, 128], [128 * V, 2], [1, VCW]]),
                        in_=lg[:],
                    )

    nc.compile()
    _prog_cache[key] = nc
    return nc


def _prep_inputs(inputs):
    f = {k: np.asarray(v) for k, v in inputs.items()}
    idx = f["idx"].astype(np.int64)
    emb = f["emb"].astype(np.float32)
    pos = f["pos_enc"].astype(np.float32)
    x_full = emb[idx] + pos[None, :, :]          # [B,T,C] f32

    scale = HD ** -0.5
    g1 = f["ln1_g"].astype(np.float32)
    b1ln = f["ln1_b"].astype(np.float32)
    g2 = f["ln2_g"].astype(np.float32)
    b2ln = f["ln2_b"].astype(np.float32)
    gf = f["lnf_g"].astype(np.float32)
    bfln = f["lnf_b"].astype(np.float32)
    W1 = f["W1"].astype(np.float32)
    Wv = f["Wv"].astype(np.float32)
    Wo = f["Wo"].astype(np.float32)
    Wlm = f["Wlm"].astype(np.float32)

    bf = lambda a: np.ascontiguousarray(a, dtype=np.float32).astype(BF16NP)
    b1_f = f["b1"].astype(np.float32) + np.einsum("lc,lcf->lf", b2ln, W1)
    bo_f = f["bo"].astype(np.float32) + np.einsum(
        "ld,ldc->lc", np.einsum("lc,lcd->ld", b1ln, Wv), Wo
    )
    blm_f = f["blm"].astype(np.float32) + bfln @ Wlm

    wq_s = f["Wq"].astype(np.float32) * scale * g1[:, :, None]
    wk_s = f["Wk"].astype(np.float32) * g1[:, :, None]
    wv_s = Wv * g1[:, :, None]
    wlm_blocks = np.ascontiguousarray(
        (Wlm * gf[:, None]).reshape(NCT, 128, NCH, VCW).transpose(2, 0, 1, 3)
    ).astype(BF16NP)

    shared = {
        "w1": bf(W1 * g2[:, :, None]),
        "w2": bf(f["W2"]),
        "b1": b1_f.astype(np.float32),
        "bo": bo_f.astype(np.float32),
        "b2": bf(f["b2"]),
        "wlm": wlm_blocks,
        "maskd": np.triu(np.ones((128, 128), dtype=np.float32)).astype(BF16NP),
    }

    x_flat = np.ascontiguousarray(x_full.reshape(B * T, C), dtype=np.float32)
    in_maps = []
    for c in range(NCORES):
        hc = slice(c * 128, (c + 1) * 128)
        im = dict(shared)
        # core c owns flat tokens [c*512, (c+1)*512) — seq c//2, half c%2 —
        # so gathered-hT chunk r is exactly rank r's contiguous token block
        im["x0"] = x_flat[c * TL:(c + 1) * TL]
        im["wq"] = bf(wq_s[:, :, hc])
        im["wk"] = bf(wk_s[:, :, hc])
        im["wv"] = bf(wv_s[:, :, hc])
        im["wo"] = bf(Wo[:, hc, :])
        in_maps.append(im)
    return in_maps, blm_f


def kernel(**inputs):
    nc = _build()
    in_maps, blm_f = _prep_inputs(inputs)
    res = run_bass_kernel_spmd(nc, in_maps, list(range(NCORES)))
    full = np.zeros((B * T, V), dtype=np.float32)
    for c in range(NCORES):
        full[c * TL:(c + 1) * TL, :] = np.asarray(
            res.results[c]["logits"], dtype=np.float32
        )
    full += blm_f[None, :]
    return full.reshape(B, T, V)


# revision 29
# speedup vs baseline: 1.2573x; 1.0886x over previous
"""GPT forward pass on 8 Trainium2 NeuronCores — v2 (head-sharded attention).

Trunk (LN/FFN/residual/lm_head) is token-parallel: core c owns token tile c
(128 tokens) of each of the 4 sequences. Attention is Megatron head-sharded:
core c owns global heads {2c, 2c+1} and computes full causal attention for
those heads over all 4096 tokens — every (head, seq) unit has the identical
causal block structure (q-tile qt needs qt+1 k-tiles), so the SPMD program
is uniform across cores while skipping all fully-masked blocks (36/64).

Per layer: LN1 -> AllGather h^T (1 MB payload) -> Q/K/V for my 2 heads over
all tokens (SBUF-resident, no DRAM round-trip) -> causal attention ->
row-parallel Wo partial -> bf16 ReduceScatter back to token owners ->
residual -> LN2 -> FFN (token-parallel). lm_head is token-sharded: each
core streams the full lnf-folded Wlm and emits bf16 logits for its 512
tokens; blm is added on the host.

LN gains are folded into consuming weights host-side (exact); ln2/lnf
biases into b1/blm (exact); projection bias b2 enters the PSUM via a K=1
ones-row matmul; bo is added after the ReduceScatter. All matmuls bf16
with fp32 PSUM; residual stream and softmax stats stay fp32. Softmax is
transposed-layout with denominators from a ones-column of V (no max
subtraction needed at these scales — matches the reference to ~5e-3).
"""

import os
import sys

for _p in ("/opt/trn_rl_repo",):
    if os.path.isdir(_p) and _p not in sys.path:
        sys.path.insert(0, _p)

import numpy as np
import ml_dtypes

BF16NP = ml_dtypes.bfloat16

import concourse.bass as bass
import concourse.mybir as mybir
import concourse.tile as tile
from concourse import bacc
from concourse.bass_utils import run_bass_kernel_spmd
from concourse.masks import make_identity

F32 = mybir.dt.float32
BF = mybir.dt.bfloat16
AF = mybir.ActivationFunctionType

V, C, T, H, L, B = 32000, 1024, 1024, 16, 4, 4
HD = C // H          # 64
FF = 4 * C           # 4096
NCORES = 8
TL = 512             # local tokens per core (4 seqs x 128)
TT = B * T // 1      # 4096 total tokens (seq-major: t = s*1024 + pos)
SEQ = B
NT = TL // 128       # 4 local t-tiles; tile tt = seq tt
NCT = C // 128       # 8 c-tiles
NFT = FF // 128      # 32 f-tiles
NTB = TT // 128      # 32 global t-blocks
NCH = 64             # vocab chunks
VCW = V // NCH       # 500
LN_EPS = 1e-5

_prog_cache = {}


def _ap(t, offset, pattern):
    return bass.AP(tensor=t.tensor if isinstance(t, bass.AP) else t, offset=offset, ap=pattern)


def _build(LL=L, sim=False):
    key = (LL, sim)
    if key in _prog_cache:
        return _prog_cache[key]

    nc = bacc.Bacc("TRN2", target_bir_lowering=False, debug=False, num_devices=NCORES)

    x0 = nc.dram_tensor("x0", [TL, C], F32, kind="ExternalInput")
    mask_d = nc.dram_tensor("maskd", [128, 128], BF, kind="ExternalInput")
    wq_d = nc.dram_tensor("wq", [L, C, 128], BF, kind="ExternalInput")   # my head cols
    wk_d = nc.dram_tensor("wk", [L, C, 128], BF, kind="ExternalInput")
    wv_d = nc.dram_tensor("wv", [L, C, 128], BF, kind="ExternalInput")
    wo_d = nc.dram_tensor("wo", [L, C, C], BF, kind="ExternalInput")
    w1_d = nc.dram_tensor("w1", [L, C, FF], BF, kind="ExternalInput")
    w2_d = nc.dram_tensor("w2", [L, FF, C], BF, kind="ExternalInput")
    b1_d = nc.dram_tensor("b1", [L, FF], F32, kind="ExternalInput")
    bo_d = nc.dram_tensor("bo", [L, C], BF, kind="ExternalInput")
    b2_d = nc.dram_tensor("b2", [L, C], BF, kind="ExternalInput")
    wlm_d = nc.dram_tensor("wlm", [NCH, NCT, 128, VCW], BF, kind="ExternalInput")

    logits_d = nc.dram_tensor("logits", [TL, V], BF, kind="ExternalOutput")

    HTSZ = C * TL            # elems in one core's hT payload (1 MB bf16)

    with tile.TileContext(nc) as tc:
        import contextlib

        with contextlib.ExitStack() as ctx:
            # SBUF pools (~per-partition KB)
            const = ctx.enter_context(tc.tile_pool(name="const", bufs=1))      # .6
            xpool = ctx.enter_context(tc.tile_pool(name="x", bufs=1))          # 16
            hpool = ctx.enter_context(tc.tile_pool(name="h", bufs=5))          # 10
            tpool = ctx.enter_context(tc.tile_pool(name="hT", bufs=1))         # 8
            big = ctx.enter_context(tc.tile_pool(name="big", bufs=2))          # 64
            qkt = ctx.enter_context(tc.tile_pool(name="qkt", bufs=1))          # 16
            vsb_p = ctx.enter_context(tc.tile_pool(name="vsb", bufs=1))        # 8.3
            otm_p = ctx.enter_context(tc.tile_pool(name="otm", bufs=1))        # 8
            w4 = ctx.enter_context(tc.tile_pool(name="w4", bufs=4))            # 16
            qwp = ctx.enter_context(tc.tile_pool(name="qw", bufs=1))           # 6
            gbpool = ctx.enter_context(tc.tile_pool(name="gb", bufs=1))        # 4.5
            misc = ctx.enter_context(tc.tile_pool(name="misc", bufs=2))        # .6
            pt_pool = ctx.enter_context(tc.tile_pool(name="pt", bufs=3))       # 6
            oraw_pool = ctx.enter_context(tc.tile_pool(name="oraw", bufs=2))   # 8
            os_pool = ctx.enter_context(tc.tile_pool(name="oS", bufs=2))       # 4
            rcp_pool = ctx.enter_context(tc.tile_pool(name="rcp", bufs=2))     # 8
            rb_pool = ctx.enter_context(tc.tile_pool(name="rb", bufs=2))       # 8
            pd_pool = ctx.enter_context(tc.tile_pool(name="pd", bufs=3))       # 6
            rcv_pool = ctx.enter_context(tc.tile_pool(name="rcv", bufs=2))     # 4
            lgout = ctx.enter_context(tc.tile_pool(name="lgout", bufs=2))      # 8
            ps_acc = ctx.enter_context(tc.tile_pool(name="psacc", bufs=4, space="PSUM"))
            ps_st = ctx.enter_context(tc.tile_pool(name="psst", bufs=2, space="PSUM"))
            ps_ov = ctx.enter_context(tc.tile_pool(name="psov", bufs=2, space="PSUM"))
            dram = ctx.enter_context(tc.tile_pool(name="dram", bufs=1, space="DRAM"))

            ident = const.tile([128, 128], BF, name="ident")
            make_identity(nc, ident)
            eps_t = const.tile([128, 1], F32, name="eps")
            nc.vector.memset(eps_t[:], LN_EPS)
            mask_t = const.tile([128, 128], BF, name="mask")
            nc.sync.dma_start(out=mask_t[:], in_=mask_d[:])
            ones1 = const.tile([1, 128], BF, name="ones1")
            nc.vector.memset(ones1[:], 1.0)

            hT_loc = dram.tile([HTSZ], BF, name="hT_loc")
            o_loc = dram.tile([NCORES * 128 * 512], BF, name="o_loc")

            # persistent residual stream fp32: tile tt = seq tt
            x_t = [xpool.tile([128, C], F32, tag=f"x{tt}", name=f"x{tt}") for tt in range(NT)]
            for tt in range(NT):
                nc.sync.dma_start(out=x_t[tt][:], in_=x0[tt * 128:(tt + 1) * 128, :])

            def emit_ln():
                """x_t -> (x-m)*rstd bf16, transposed hT tiles (g/b folded away)."""
                h_tiles = []
                for tt in range(NT):
                    stats = misc.tile([128, 2, 6], F32, name="stats", tag="stats")
                    xv = x_t[tt][:].rearrange("p (s d) -> p s d", s=2)
                    nc.vector.bn_stats(out=stats[:, 0, :], in_=xv[:, 0, :])
                    nc.vector.bn_stats(out=stats[:, 1, :], in_=xv[:, 1, :])
                    mv = misc.tile([128, 2], F32, name="mv", tag="mv")
                    nc.vector.bn_aggr(out=mv[:], in_=stats[:])
                    rstd = misc.tile([128, 1], F32, name="rstd", tag="rstd")
                    nc.scalar.activation(rstd[:], mv[:, 1:2], AF.Sqrt, bias=eps_t[:])
                    nc.vector.reciprocal(rstd[:], rstd[:])
                    h = hpool.tile([128, C], BF, tag="h", name="h")
                    nc.vector.tensor_scalar(
                        out=h[:], in0=x_t[tt][:], scalar1=mv[:, 0:1], scalar2=rstd[:],
                        op0=mybir.AluOpType.subtract, op1=mybir.AluOpType.mult,
                    )
                    h_tiles.append(h)
                hT_all = tpool.tile([128, NCT, 512], BF, tag="hTall", name="hTall")
                for ct in range(NCT):
                    pst = ps_st.tile([128, 512], BF, tag="st", name="pst")
                    for tt in range(NT):
                        nc.tensor.transpose(
                            pst[:, tt * 128:(tt + 1) * 128],
                            h_tiles[tt][:, ct * 128:(ct + 1) * 128],
                            ident[:],
                        )
                    nc.vector.tensor_copy(out=hT_all[:, ct, :], in_=pst[:])
                return hT_all

            for l in range(LL):
                lw = l % L
                # per-layer bias/const tiles (issued early; Pool queue quiet)
                b2_r = gbpool.tile([1, C], BF, tag="b2r", name="b2_r")
                nc.sync.dma_start(out=b2_r[:], in_=b2_d[lw:lw + 1, :])
                bo_r = gbpool.tile([1, C], BF, tag="bor", name="bo_r")
                nc.sync.dma_start(out=bo_r[:], in_=bo_d[lw:lw + 1, :])
                b1_t = misc.tile([128, NFT], F32, tag="b1", name="b1_t")
                nc.gpsimd.dma_start(
                    out=b1_t[:], in_=_ap(b1_d, lw * FF, [[1, 128], [128, NFT]])
                )
                # my-head projection weights [128c-in-ct, ct, 128d] — one DMA each
                wq_t = qwp.tile([128, NCT, 128], BF, tag="qw", name="wq_t")
                wk_t = qwp.tile([128, NCT, 128], BF, tag="kw", name="wk_t")
                wv_t = qwp.tile([128, NCT, 128], BF, tag="vw", name="wv_t")
                for wt, wd in ((wq_t, wq_d), (wk_t, wk_d), (wv_t, wv_d)):
                    nc.sync.dma_start(
                        out=wt[:],
                        in_=_ap(wd[0:1, 0:1, 0:1], lw * C * 128,
                                [[128, 128], [128 * 128, NCT], [1, 128]]),
                    )

                # ---- LN1 -> hT, publish (one DMA), AllGather ----
                hT = emit_ln()
                nc.sync.dma_start(
                    out=_ap(hT_loc, 0, [[512, 128], [128 * 512, NCT], [1, 512]]),
                    in_=hT[:],
                )
                hT_full = dram.tile(
                    [NCORES * HTSZ], BF,
                    addr_space="Local" if sim else "Shared", name=f"hT_full{l}",
                )
                if sim:
                    nc.sync.dma_start(
                        out=_ap(hT_full, 0, [[2048, HTSZ // 2048], [1, 2048]]),
                        in_=_ap(hT_loc, 0, [[2048, HTSZ // 2048], [1, 2048]]),
                    )
                else:
                    nc.gpsimd.collective_compute(
                        "AllGather",
                        mybir.AluOpType.bypass,
                        replica_groups=[list(range(NCORES))],
                        ins=[_ap(hT_loc, 0, [[2048, HTSZ // 2048], [1, 2048]])],
                        outs=[_ap(hT_full, 0, [[2048, NCORES * HTSZ // 2048], [1, 2048]])],
                    )

                # gathered h^T lands per rank-chunk so QKV(seq s) starts after
                # 2 chunks, and attention(s) overlaps QKV(s+1)
                hT_sb = [None, None]

                def load_chunk(ch):
                    half = ch // 4
                    if hT_sb[half] is None:
                        hT_sb[half] = big.tile(
                            [128, 4, NCT, 512], BF, tag="big", name=f"hTsb{half}"
                        )
                    nc.sync.dma_start(
                        out=hT_sb[half][:, ch % 4, :, :],
                        in_=_ap(hT_full, ch * HTSZ, [[512, 128], [128 * 512, NCT], [1, 512]]),
                    )

                def htf(ct, ch):
                    return hT_sb[ch // 4][:, ch % 4, ct, :]

                qT_s, kT_s, v_ss, oTm_s = [], [], [], []
                for s in range(SEQ):
                    qT_s.append(qkt.tile([128, 1024], BF, tag=f"qT{s}", name=f"qT{s}"))
                    kT_s.append(qkt.tile([128, 1024], BF, tag=f"kT{s}", name=f"kT{s}"))
                    v_ss.append(vsb_p.tile([128, 8, 2, HD + 1], BF, tag=f"v{s}", name=f"v{s}"))
                    oTm_s.append(otm_p.tile([128, 1024], BF, tag=f"oTm{s}", name=f"oTm{s}"))

                for s in range(SEQ):
                    for hh in range(2):
                        load_chunk(2 * s + hh)
                    # K^T then Q^T for this seq (2 chunks each)
                    for dst, wt_l in ((kT_s[s], wk_t), (qT_s[s], wq_t)):
                        for hh in range(2):
                            ch = 2 * s + hh
                            ps = ps_acc.tile([128, 512], F32, tag="acc", name="acc")
                            for ct in range(NCT):
                                nc.tensor.matmul(
                                    ps[:], wt_l[:, ct, :], htf(ct, ch),
                                    start=(ct == 0), stop=(ct == NCT - 1),
                                )
                            nc.vector.tensor_copy(
                                out=dst[:, hh * 512:(hh + 1) * 512], in_=ps[:]
                            )
                    # V natural [t, my 128 d] with ones column
                    v_sb = v_ss[s]
                    nc.vector.memset(v_sb[:, :, :, HD:HD + 1], 1.0)
                    for i in range(8):
                        tb = s * 8 + i
                        ps = ps_acc.tile([128, 128], F32, tag="acc", name="psv")
                        for ct in range(NCT):
                            nc.tensor.matmul(
                                ps[:], htf(ct, tb // 4)[:, (tb % 4) * 128:(tb % 4 + 1) * 128],
                                wv_t[:, ct, :],
                                start=(ct == 0), stop=(ct == NCT - 1),
                            )
                        nc.vector.tensor_copy(out=v_sb[:, i, 0, 0:HD], in_=ps[:, 0:HD])
                        nc.vector.tensor_copy(out=v_sb[:, i, 1, 0:HD], in_=ps[:, HD:2 * HD])

                    # ---- causal attention for this seq, both heads ----
                    oraw = [
                        oraw_pool.tile([HD + 1, 1024], F32, tag="oraw", name="oraw")
                        for _ in range(2)
                    ]
                    for qt in range(8):
                        kept = qt + 1
                        for hp in range(2):
                            poff = hp * HD
                            q_sl = qT_s[s][poff:poff + HD, qt * 128:(qt + 1) * 128]
                            pT = pt_pool.tile([128, 8, 128], BF, tag="pt", name="pt")
                            for half in range((kept + 3) // 4):
                                cnt = min(4, kept - half * 4)
                                st = ps_st.tile([128, 4, 128], F32, tag="st", name="st")
                                for k4 in range(cnt):
                                    kt = half * 4 + k4
                                    nc.tensor.matmul(
                                        st[:, k4, :],
                                        kT_s[s][poff:poff + HD, kt * 128:(kt + 1) * 128],
                                        q_sl, start=True, stop=True,
                                    )
                                nc.scalar.activation(
                                    pT[:, half * 4:half * 4 + cnt, :], st[:, 0:cnt, :], AF.Exp
                                )
                            nc.vector.tensor_mul(
                                out=pT[:, qt, :], in0=pT[:, qt, :], in1=mask_t[:]
                            )
                            ov = ps_ov.tile([128, 128], F32, tag="ov", name="ov")
                            for i in range(kept):
                                nc.tensor.matmul(
                                    ov[0:HD + 1, :], v_sb[:, i, hp, :], pT[:, i, :],
                                    start=(i == 0), stop=(i == kept - 1),
                                )
                            nc.vector.tensor_copy(
                                out=oraw[hp][:, qt * 128:(qt + 1) * 128], in_=ov[0:HD + 1, :]
                            )
                    for hp in range(2):
                        recips = rcp_pool.tile([1, 1024], F32, tag="recips", name="recips")
                        nc.vector.reciprocal(recips[:], oraw[hp][HD:HD + 1, :])
                        rc_b = dram.tile([1024], F32, name=f"rcb{l}_{s}_{hp}")
                        nc.sync.dma_start(out=rc_b[:], in_=recips[:])
                        rb = rb_pool.tile([HD, 1024], F32, tag="rb", name="rb")
                        nc.gpsimd.dma_start(out=rb[:], in_=_ap(rc_b, 0, [[0, HD], [1, 1024]]))
                        if hp == 0:
                            nc.gpsimd.tensor_mul(
                                out=oTm_s[s][0:HD, :], in0=oraw[hp][0:HD, :], in1=rb[:]
                            )
                        else:
                            oS = os_pool.tile([HD, 1024], BF, tag="oS", name="oS")
                            nc.gpsimd.tensor_mul(out=oS[:], in0=oraw[hp][0:HD, :], in1=rb[:])
                            nc.sync.dma_start(out=oTm_s[s][HD:128, :], in_=oS[:])
                    nc.sync.dma_start(
                        out=_ap(o_loc, 2 * s * 128 * 512, [[512, 128], [128 * 512, 2], [1, 512]]),
                        in_=oTm_s[s][:],
                    )
                o_recv = dram.tile([NCORES * 128 * 512], BF, name=f"orecv{l}")
                if sim:
                    nc.sync.dma_start(
                        out=_ap(o_recv, 0, [[2048, NCORES * 128 * 512 // 2048], [1, 2048]]),
                        in_=_ap(o_loc, 0, [[2048, NCORES * 128 * 512 // 2048], [1, 2048]]),
                    )
                else:
                    nc.gpsimd.collective_compute(
                        "AllToAll",
                        mybir.AluOpType.bypass,
                        replica_groups=[list(range(NCORES))],
                        ins=[_ap(o_loc, 0, [[2048, NCORES * 128 * 512 // 2048], [1, 2048]])],
                        outs=[_ap(o_recv, 0, [[2048, NCORES * 128 * 512 // 2048], [1, 2048]])],
                    )
                orv = otm_p.tile([128, NCT, 512], BF, tag="orv", name="orv")
                nc.sync.dma_start(
                    out=orv[:],
                    in_=_ap(o_recv, 0, [[512, 128], [128 * 512, NCT], [1, 512]]),
                )
                for nf in range(2):
                    pss = [ps_acc.tile([128, 512], F32, tag="acc", name="acc") for _ in range(4)]
                    for cg in range(2):
                        wt = w4.tile([128, 4, 512], BF, tag="w", name="wot")
                        nc.sync.dma_start(
                            out=wt[:],
                            in_=_ap(wo_d[0:1, 0:1, 0:1],
                                    lw * C * C + cg * 512 * C + nf * 512,
                                    [[C, 128], [128 * C, 4], [1, 512]]),
                        )
                        for ci in range(4):
                            ct = cg * 4 + ci
                            for tt in range(NT):
                                nc.tensor.matmul(
                                    pss[tt][:], orv[:, ct, tt * 128:(tt + 1) * 128], wt[:, ci, :],
                                    start=(ct == 0), stop=False,
                                )
                    for tt in range(NT):
                        nc.tensor.matmul(
                            pss[tt][:], ones1[:], bo_r[:, nf * 512:(nf + 1) * 512],
                            start=False, stop=True,
                        )
                        xs = x_t[tt][:, nf * 512:(nf + 1) * 512]
                        nc.vector.tensor_add(out=xs, in0=xs, in1=pss[tt][:])

                # ---- FFN (token-parallel, biases in-psum / in-activation) ----
                h2T = emit_ln()
                ug = big.tile([128, NFT, 512], BF, tag="big", name="ug")
                for fg in range(8):
                    pss = [ps_acc.tile([128, 512], F32, tag="acc", name="acc") for _ in range(4)]
                    for hf in range(2):
                        wt = w4.tile([128, 4, 512], BF, tag="w", name="w1t")
                        nc.sync.dma_start(
                            out=wt[:],
                            in_=_ap(w1_d[0:1, 0:1, 0:1],
                                    lw * C * FF + hf * 512 * FF + fg * 512,
                                    [[FF, 128], [128 * FF, 4], [1, 512]]),
                        )
                        for ci in range(4):
                            ct = hf * 4 + ci
                            for f4 in range(4):
                                nc.tensor.matmul(
                                    pss[f4][:], wt[:, ci, f4 * 128:(f4 + 1) * 128], h2T[:, ct, :],
                                    start=(ct == 0), stop=(ct == NCT - 1),
                                )
                    for f4 in range(4):
                        ft = fg * 4 + f4
                        nc.scalar.activation(
                            ug[:, ft, :], pss[f4][:], AF.Gelu, bias=b1_t[:, ft:ft + 1]
                        )
                for nf in range(2):
                    pss = [ps_acc.tile([128, 512], F32, tag="acc", name="acc") for _ in range(4)]
                    for g8 in range(8):
                        wt = w4.tile([128, 4, 512], BF, tag="w", name="w2t")
                        nc.sync.dma_start(
                            out=wt[:],
                            in_=_ap(w2_d[0:1, 0:1, 0:1],
                                    lw * FF * C + g8 * 512 * C + nf * 512,
                                    [[C, 128], [128 * C, 4], [1, 512]]),
                        )
                        for fi in range(4):
                            ft = g8 * 4 + fi
                            for tt in range(NT):
                                nc.tensor.matmul(
                                    pss[tt][:], ug[:, ft, tt * 128:(tt + 1) * 128], wt[:, fi, :],
                                    start=(ft == 0), stop=False,
                                )
                    for tt in range(NT):
                        nc.tensor.matmul(
                            pss[tt][:], ones1[:], b2_r[:, nf * 512:(nf + 1) * 512],
                            start=False, stop=True,
                        )
                        xs = x_t[tt][:, nf * 512:(nf + 1) * 512]
                        nc.vector.tensor_add(out=xs, in0=xs, in1=pss[tt][:])

            # ---- final LN (folded) + token-sharded lm_head ----
            hfT = emit_ln()
            for ch in range(NCH):
                wlm_c = big.tile([128, NCT, VCW], BF, tag="big", name="wlm_c")
                nc.sync.dma_start(
                    out=wlm_c[:],
                    in_=_ap(wlm_d[0:1, 0:1, 0:1, 0:1], ch * NCT * 128 * VCW,
                            [[VCW, 128], [128 * VCW, NCT], [1, VCW]]),
                )
                for th in range(2):
                    lg = lgout.tile([128, 2, VCW], BF, tag="lg", name="lg")
                    for ti in range(2):
                        tt = th * 2 + ti
                        ps = ps_acc.tile([128, VCW], F32, tag="acc", name="acc")
                        for ct in range(NCT):
                            nc.tensor.matmul(
                                ps[:], hfT[:, ct, tt * 128:(tt + 1) * 128], wlm_c[:, ct, :],
                                start=(ct == 0), stop=(ct == NCT - 1),
                            )
                        if ti % 2 == 0:
                            nc.vector.tensor_copy(out=lg[:, ti, :], in_=ps[:])
                        else:
                            nc.scalar.activation(lg[:, ti, :], ps[:], AF.Copy)
                    nc.sync.dma_start(
                        out=_ap(logits_d[0:1, 0:1], th * 2 * 128 * V + ch * VCW,
                                [[V, 128], [128 * V, 2], [1, VCW]]),
                        in_=lg[:],
                    )

    nc.compile()
    _prog_cache[key] = nc
    return nc


def _prep_inputs(inputs):
    f = {k: np.asarray(v) for k, v in inputs.items()}
    idx = f["idx"].astype(np.int64)
    emb = f["emb"].astype(np.float32)
    pos = f["pos_enc"].astype(np.float32)
    x_full = emb[idx] + pos[None, :, :]          # [B,T,C] f32

    scale = HD ** -0.5
    g1 = f["ln1_g"].astype(np.float32)
    b1ln = f["ln1_b"].astype(np.float32)
    g2 = f["ln2_g"].astype(np.float32)
    b2ln = f["ln2_b"].astype(np.float32)
    gf = f["lnf_g"].astype(np.float32)
    bfln = f["lnf_b"].astype(np.float32)
    W1 = f["W1"].astype(np.float32)
    Wv = f["Wv"].astype(np.float32)
    Wo = f["Wo"].astype(np.float32)
    Wlm = f["Wlm"].astype(np.float32)

    bf = lambda a: np.ascontiguousarray(a, dtype=np.float32).astype(BF16NP)
    b1_f = f["b1"].astype(np.float32) + np.einsum("lc,lcf->lf", b2ln, W1)
    bo_f = f["bo"].astype(np.float32) + np.einsum(
        "ld,ldc->lc", np.einsum("lc,lcd->ld", b1ln, Wv), Wo
    )
    blm_f = f["blm"].astype(np.float32) + bfln @ Wlm

    wq_s = f["Wq"].astype(np.float32) * scale * g1[:, :, None]
    wk_s = f["Wk"].astype(np.float32) * g1[:, :, None]
    wv_s = Wv * g1[:, :, None]
    wlm_blocks = np.ascontiguousarray(
        (Wlm * gf[:, None]).reshape(NCT, 128, NCH, VCW).transpose(2, 0, 1, 3)
    ).astype(BF16NP)

    shared = {
        "w1": bf(W1 * g2[:, :, None]),
        "w2": bf(f["W2"]),
        "b1": b1_f.astype(np.float32),
        "bo": bf(bo_f),
        "wo": bf(Wo),
        "b2": bf(f["b2"]),
        "wlm": wlm_blocks,
        "maskd": np.triu(np.ones((128, 128), dtype=np.float32)).astype(BF16NP),
    }

    x_flat = np.ascontiguousarray(x_full.reshape(B * T, C), dtype=np.float32)
    in_maps = []
    for c in range(NCORES):
        hc = slice(c * 128, (c + 1) * 128)
        im = dict(shared)
        # core c owns flat tokens [c*512, (c+1)*512) — seq c//2, half c%2 —
        # so gathered-hT chunk r is exactly rank r's contiguous token block
        im["x0"] = x_flat[c * TL:(c + 1) * TL]
        im["wq"] = bf(wq_s[:, :, hc])
        im["wk"] = bf(wk_s[:, :, hc])
        im["wv"] = bf(wv_s[:, :, hc])
        in_maps.append(im)
    return in_maps, blm_f


def kernel(**inputs):
    nc = _build()
    in_maps, blm_f = _prep_inputs(inputs)
    res = run_bass_kernel_spmd(nc, in_maps, list(range(NCORES)))
    full = np.zeros((B * T, V), dtype=np.float32)
    for c in range(NCORES):
        full[c * TL:(c + 1) * TL, :] = np.asarray(
            res.results[c]["logits"], dtype=np.float32
        )
    full += blm_f[None, :]
    return full.reshape(B, T, V)
